# revision 1
# baseline (speedup 1.0000x reference)
"""CosAttn2d (cosFormer-style 2D linear attention) Trainium2 kernel.

Problem (hardcoded shapes): q,k,v [32, 512, 32, 32] f32, n_head=8, M=48.
Sharding: data-parallel over batch across 8 cores (4 batches each).

Math per (batch, head h, branch c in 4 cos/sin branches):
    qa = relu(q * C**-0.25) + 1e-5, ka likewise
    qf_c = qa * coef_c, kf_c = ka * coef_c           (coef over spatial n)
    ctx_c = kf_c^T @ vf        [64, 64]   (contract over n = 1024 tokens)
    out = sum_c qf_c @ ctx_c ; D = sum_c qf_c . ksum_c ; out /= D

Device layout (per core, per batch):
  - q natural channel-major [512ch, 1024n] bf16 (4 partition blocks = head
    PAIRS); k, v pre-transposed on host to spatial-major [1024n, 512ch] bf16.
  - ctx via PE: lhsT = km (coef-scaled k, spatial-major), rhs = [v | ones]
    -> psum [128 m-pair, 129] per (pair, branch); col 128 = ksum.
  - out^T via PE: lhsT = block-diag [ctx_h0, ctx_h1] [128, 128], rhs = qm
    (coef-scaled q, channel-major) [128, 512] -> accumulate 4 branches.
  - D via small matmuls: kD (ksum columns) x qr -> s; coef-combine; recip;
    broadcast D_inv via DRAM bounce; normalize; store bf16.

Host converts inputs to bf16 / transposed layouts and the output back to f32
(layout/dtype prep only - all arithmetic on device).
"""

import sys

sys.path.insert(0, "/opt/trn_rl_repo")

from contextlib import ExitStack
from math import pi

import numpy as np
import ml_dtypes

from concourse import bass, mybir, tile
from concourse.bass_utils import run_bass_kernel_spmd

F32 = mybir.dt.float32
BF16 = mybir.dt.bfloat16
BFNP = ml_dtypes.bfloat16

B, C, H, W = 32, 512, 32, 32
NHEAD, MVAL = 8, 48
N = H * W            # 1024 tokens
NCORES = 8
BPC = B // NCORES    # batches per core
DN = float(C) ** -0.25
EPS = 1e-5

MULT = mybir.AluOpType.mult
ADD = mybir.AluOpType.add
MAX = mybir.AluOpType.max
COPY = mybir.ActivationFunctionType.Copy


def _fix_waits(nc):
    """This walrus build rejects sync waits on CTRL_NO-struct instructions
    (Drain/NoOp) and allows at most one on DMACopy. Hoist the excess into
    standalone InstEventSemaphore instructions on the same engine (the
    sequencer executes them in order, so semantics are unchanged)."""
    for fn in nc.m.functions:
        for bb in fn.blocks:
            newlist = []
            for inst in bb.instructions:
                si = inst.sync_info
                if isinstance(inst, mybir.InstEventSemaphore):
                    cap = 1
                elif isinstance(inst, (mybir.InstDrain, mybir.InstNoOp)):
                    cap = 0
                else:
                    cap = 1
                if si is not None and len(si.on_wait) > cap:
                    waits = list(si.on_wait)
                    for w in waits[cap:]:
                        ev = mybir.InstEventSemaphore(
                            name=nc.get_next_instruction_name(), ins=[], outs=[])
                        ev.engine = inst.engine
                        ev.sync_info = mybir.SyncInfo(on_wait=[w], on_update=[])
                        nc.register_instruction(ev, overwrite=True)
                        newlist.append(ev)
                    inst.sync_info = mybir.SyncInfo(
                        on_wait=waits[:cap], on_update=list(si.on_update))
                newlist.append(inst)
            bb.instructions[:] = newlist


def build_nc():
    nc = bass.Bass()

    qn_d = nc.declare_dram_parameter("qn", [BPC, C, N], BF16, isOutput=False)
    ks_d = nc.declare_dram_parameter("ks", [BPC, N, C], BF16, isOutput=False)
    vs_d = nc.declare_dram_parameter("vs", [BPC, N, C], BF16, isOutput=False)
    coefq_d = nc.declare_dram_parameter("coefq", [128, 4, N], BF16, isOutput=False)
    coefk_d = nc.declare_dram_parameter("coefk", [128, 8, 4], F32, isOutput=False)
    coefke_d = nc.declare_dram_parameter("coefke", [128, 8, 4], F32, isOutput=False)
    coefkb_d = nc.declare_dram_parameter("coefkb", [128, 8, 4], BF16, isOutput=False)
    coefD_d = nc.declare_dram_parameter("coefD", [128, N], BF16, isOutput=False)
    dcomb_d = nc.declare_dram_parameter("dcomb", [128, 32], BF16, isOutput=False)
    ident_d = nc.declare_dram_parameter("ident", [128, 128], BF16, isOutput=False)
    epsSc_d = nc.declare_dram_parameter("epsSc", [4, 1], F32, isOutput=False)
    o_d = nc.declare_dram_parameter("o", [BPC, C, N], BF16, isOutput=True)

    # eps correction for ksum: sum_n coef_c(n) * eps, folded into kD assembly
    from math import cos as _cos, sin as _sin
    idx = [i * pi / (2 * MVAL) for i in range(H)]
    cvec = [_cos(x) for x in idx]
    svec = [_sin(x) for x in idx]
    sc, ss = sum(cvec), sum(svec)
    epsS = [float(EPS * a * b) for a, b in
            [(sc, sc), (sc, ss), (ss, sc), (ss, ss)]]

    with tile.TileContext(nc) as tc, ExitStack() as ctx:
        const = ctx.enter_context(tc.tile_pool(name="const", bufs=1))
        stat = ctx.enter_context(tc.tile_pool(name="stat", bufs=1))
        io = ctx.enter_context(tc.tile_pool(name="io", bufs=2))
        work = ctx.enter_context(tc.tile_pool(name="work", bufs=2))
        qmp = ctx.enter_context(tc.tile_pool(name="qmp", bufs=8))
        small = ctx.enter_context(tc.tile_pool(name="small", bufs=4))
        pctx = ctx.enter_context(tc.tile_pool(name="pctx", bufs=4, space="PSUM"))
        pout = ctx.enter_context(tc.tile_pool(name="pout", bufs=2, space="PSUM"))
        psd = ctx.enter_context(tc.tile_pool(name="psd", bufs=1, space="PSUM"))
        pks = ctx.enter_context(tc.tile_pool(name="pks", bufs=1, space="PSUM"))
        dram = ctx.enter_context(tc.tile_pool(name="dram", bufs=2, space="DRAM"))

        # constants
        coefq = const.tile([128, 4, N], BF16, tag="coefq", name="coefq")
        coefk = const.tile([128, 8, 4], F32, tag="coefk", name="coefk")
        coefke = const.tile([128, 8, 4], F32, tag="coefke", name="coefke")
        coefkb = const.tile([128, 8, 4], BF16, tag="coefkb", name="coefkb")
        coefD = const.tile([128, N], BF16, tag="coefD", name="coefD")
        dcomb = const.tile([128, 32], BF16, tag="dcomb", name="dcomb")
        ident = const.tile([128, 128], BF16, tag="ident", name="ident")
        epsSc = const.tile([4, 1], F32, tag="epsSc", name="epsSc")
        nc.sync.dma_start(coefq[:], coefq_d[:])
        nc.sync.dma_start(coefk[:], coefk_d[:])
        nc.sync.dma_start(coefke[:], coefke_d[:])
        nc.sync.dma_start(coefkb[:], coefkb_d[:])
        nc.sync.dma_start(coefD[:], coefD_d[:])
        nc.sync.dma_start(dcomb[:], dcomb_d[:])
        nc.sync.dma_start(ident[:], ident_d[:])
        nc.sync.dma_start(epsSc[:], epsSc_d[:])

        # persistent zero-initialized tiles (diag blocks rewritten per batch,
        # off-diag zeros never touched); double-buffered by batch parity
        L = {}
        kD = {}
        for par in range(2):
            for p in range(4):
                kD[par, p] = stat.tile([128, 32], BF16, tag=f"kD{par}{p}", name=f"kD{par}{p}")
                nc.gpsimd.memset(kD[par, p][:], 0.0)
                L[par, p] = stat.tile([128, 4, 128], BF16, tag=f"L{par}{p}", name=f"L{par}{p}")
                nc.gpsimd.memset(
                    L[par, p][:].rearrange("r c f -> r (c f)"), 0.0)

        for b in range(BPC):
            par = b % 2

            qnb = io.tile([128, 4, N], BF16, tag="qnb", name="qnb")
            nc.sync.dma_start(qnb[:], qn_d[b].rearrange("(p r) n -> r p n", r=128))
            ksr = io.tile([128, 8, C], BF16, tag="ksr", name="ksr")
            nc.sync.dma_start(ksr[:], ks_d[b].rearrange("(s r) c -> r s c", r=128))
            vsr = io.tile([128, 8, C], BF16, tag="vsr", name="vsr")
            nc.sync.dma_start(vsr[:], vs_d[b].rearrange("(s r) c -> r s c", r=128))

            # relu(dn*x) in one pass, bf16 (GPSIMD: single-src streams at
            # line rate there, freeing DVE/ACT)
            qr = work.tile([128, 4, N], BF16, tag="qr", name="qr")
            nc.vector.tensor_scalar(
                qr[:].rearrange("r p n -> r (p n)"),
                qnb[:].rearrange("r p n -> r (p n)"), DN, 0.0, MULT, MAX)
            ka = work.tile([128, 8, C], BF16, tag="ka", name="ka")
            nc.vector.tensor_scalar(
                ka[:].rearrange("r s c -> r (s c)"),
                ksr[:].rearrange("r s c -> r (s c)"), DN, 0.0, MULT, MAX)
            qa = work.tile([128, 4, N], BF16, tag="qa", name="qa")
            nc.gpsimd.tensor_scalar_add(
                qa[:].rearrange("r p n -> r (p n)"),
                qr[:].rearrange("r p n -> r (p n)"), EPS)

            # ksum_c[ch] = sum_n coef_c(n) ka(n, ch) via tiny-M matmuls,
            # then transpose to channel-major columns for the D matmuls
            ksm = pks.tile([4, 512], F32, tag="ksm", name="ksm")
            for s in range(8):
                nc.tensor.matmul(ksm[:], coefkb[:, s, :], ka[:, s, :],
                                 start=(s == 0), stop=(s == 7))
            ksb = small.tile([4, 512], BF16, tag="ksb", name="ksb")
            nc.scalar.add(ksb[:], ksm[:], epsSc[:])
            kdt = pks.tile([128, 16], BF16, tag="ksm", name="kdt")
            for p in range(4):
                nc.tensor.transpose(
                    kdt[:, 4 * p:4 * (p + 1)],
                    ksb[:, 128 * p:128 * (p + 1)], ident[0:4, 0:4])
            for p in range(4):
                for h in range(2):
                    rs = slice(64 * h, 64 * (h + 1))
                    nc.vector.tensor_copy(
                        kD[par, p][rs, 4 * h:4 * h + 4],
                        kdt[rs, 4 * p:4 * p + 4])

            # context phase: per branch, km = coef*ka + eps*coef, then
            # cps_p[:, c, :] += km_block^T @ v_pair over 8 token chunks
            cpsd = {p: pctx.tile([128, 4, 128], F32, tag="cps", name="cps")
                    for p in range(4)}
            for c in range(4):
                km = work.tile([128, 8, C], BF16, tag="km", name="km")
                for s in range(8):
                    eng = nc.vector
                    eng.tensor_scalar(
                        km[:, s, :], ka[:, s, :],
                        coefk[:, s, c:c + 1], coefke[:, s, c:c + 1],
                        MULT, ADD)
                for p in range(4):
                    for s in range(8):
                        nc.tensor.matmul(
                            cpsd[p][:, c, :], km[:, s, 128 * p:128 * (p + 1)],
                            vsr[:, s, 128 * p:128 * (p + 1)],
                            start=(s == 0), stop=(s == 7))
            # harvest block-diag ctx for the out-phase lhsT (2 copies per pair)
            for p in range(4):
                for h in range(2):
                    rs = slice(64 * h, 64 * (h + 1))
                    nc.scalar.activation(
                        L[par, p][rs, :, rs], cpsd[p][rs, :, rs], COPY)

            # q-side: qm_pc = qa * coef_c (split DVE / GPSIMD)
            qm = {}
            for p in range(4):
                for c in range(4):
                    t = qmp.tile([128, N], BF16, tag="qm", name="qm")
                    qm[p, c] = t
                    nc.gpsimd.tensor_tensor(
                        t[:], qa[:, p, :], coefq[:, c, :], MULT)

            # D: s = kD^T @ qa ; coef-combine via dcomb matmul; reciprocal
            dinv = small.tile([8, N], BF16, tag="dinv", name="dinv")
            for half in range(2):
                hs = slice(512 * half, 512 * (half + 1))
                sps = psd.tile([128, 512], F32, tag="sps", name="sps")
                for p in range(4):
                    nc.tensor.matmul(
                        sps[32 * p:32 * (p + 1), :], kD[par, p][:],
                        qa[:, p, hs], start=True, stop=True,
                        tile_position=(0, 32 * p))
                sd = small.tile([128, 512], BF16, tag="sd", name="sd")
                nc.vector.tensor_tensor(sd[:], sps[:], coefD[:, hs], MULT)
                dps = psd.tile([32, 512], F32, tag="sps", name="dps")
                nc.tensor.matmul(dps[:], dcomb[:], sd[:], start=True, stop=True)
                with nc.allow_low_precision(reason="bf16 D_inv within tolerance"):
                    nc.vector.reciprocal(dinv[0:8, hs], dps[0:8, :])

            # broadcast D_inv rows to 64 partitions each via DRAM bounce
            dvd = dram.tile([8, N], BF16, tag="dvd", name="dvd")
            nc.scalar.dma_start(dvd[:], dinv[:])

            # out phase + fused normalize + store
            for p in range(4):
                for half in range(2):
                    hs = slice(512 * half, 512 * (half + 1))
                    ops = pout.tile([128, 512], F32, tag="ops", name="ops")
                    for c in range(4):
                        nc.tensor.matmul(
                            ops[:], L[par, p][:, c, :], qm[p, c][:, hs],
                            start=(c == 0), stop=(c == 3))
                    dbc = small.tile([128, 512], BF16, tag="dbc", name="dbc")
                    nc.scalar.dma_start(
                        dbc[:],
                        dvd[2 * p:2 * p + 2, hs]
                        .partition_broadcast(64).rearrange("r h n -> h r n"))
                    onrm = small.tile([128, 512], BF16, tag="onrm", name="onrm")
                    nc.vector.tensor_tensor(onrm[:], ops[:], dbc[:], MULT)
                    nc.scalar.dma_start(o_d[b, 128 * p:128 * (p + 1), hs], onrm[:])

    _fix_waits(nc)
    return nc


_NC = None


def _get_nc():
    global _NC
    if _NC is None:
        _NC = build_nc()
    return _NC


def _host_prep(q, k, v, n_head, M):
    n_head = int(n_head)
    M = int(M)
    assert q.shape == (B, C, H, W) and n_head == NHEAD
    dn32 = np.float32(DN)

    idx = np.arange(H, dtype=np.float32)
    freq = np.float32(pi / (2 * M))
    co, si = np.cos(idx * freq), np.sin(idx * freq)
    coef = np.stack([
        np.outer(co, co), np.outer(co, si),
        np.outer(si, co), np.outer(si, si)]).reshape(4, N).astype(np.float32)

    coefq = np.broadcast_to(coef[None], (128, 4, N)).astype(BFNP)
    # coefk[r, s, c] = coef[c, s*128 + r]
    coefk = np.ascontiguousarray(
        coef.reshape(4, 8, 128).transpose(2, 1, 0)).astype(np.float32)
    coefke = (coefk * np.float32(EPS)).astype(np.float32)
    coefD = np.zeros((128, N), np.float32)
    dcomb = np.zeros((128, 32), np.float32)
    for p in range(4):
        for h in range(2):
            for c in range(4):
                coefD[32 * p + 4 * h + c] = coef[c]
                dcomb[32 * p + 4 * h + c, 2 * p + h] = 1.0
    coefD = coefD.astype(BFNP)
    dcomb = dcomb.astype(BFNP)
    coefkb = coefk.astype(BFNP)
    ident = np.eye(128, dtype=np.float32).astype(BFNP)
    sc, ss = co.sum(), si.sum()
    epsSc = (np.float32(EPS) * np.array(
        [sc * sc, sc * ss, ss * sc, ss * ss], np.float32)).reshape(4, 1)

    qf = q.reshape(B, C, N).astype(BFNP)
    kf = np.ascontiguousarray(
        k.reshape(B, C, N).transpose(0, 2, 1)).astype(BFNP)
    vf = np.ascontiguousarray(
        v.reshape(B, C, N).transpose(0, 2, 1)).astype(BFNP)

    in_maps = []
    for core in range(NCORES):
        b0 = core * BPC
        in_maps.append({
            "qn": qf[b0:b0 + BPC], "ks": kf[b0:b0 + BPC], "vs": vf[b0:b0 + BPC],
            "coefq": coefq, "coefk": coefk, "coefke": coefke,
            "coefkb": coefkb, "ident": ident, "epsSc": epsSc,
            "coefD": coefD, "dcomb": dcomb,
        })
    return in_maps


def run(q, k, v, n_head=8, M=48, trace=False):
    nc = _get_nc()
    in_maps = _host_prep(q, k, v, n_head, M)
    res = run_bass_kernel_spmd(nc, in_maps, core_ids=list(range(NCORES)),
                               trace=trace)
    outs = []
    for core in range(NCORES):
        o = np.asarray(res.results[core]["o"]).astype(np.float32)
        outs.append(o.reshape(BPC, C, H, W))
    return np.concatenate(outs, axis=0), res


def kernel(q, k, v, n_head=8, M=48):
    out, _ = run(q, k, v, n_head, M)
    return out



# revision 15
# speedup vs baseline: 1.5422x; 1.5422x over previous
"""CosAttn2d (cosFormer-style 2D linear attention) Trainium2 kernel.

Problem (hardcoded shapes): q,k,v [32, 512, 32, 32] f32, n_head=8, M=48.
Sharding: data-parallel over batch across 8 cores (4 batches each).

Math per (batch, head h, branch c=(a,b) of 4 cos/sin branches):
    coef_c(n) = t_a(r_n) * t_b(u_n), token n=(r,u) on the 32x32 grid
    qf_c = relu(q*dn) * coef_c ; kf_c likewise (eps terms dropped: O(1e-4) rel)
    ctx_c = kf_c^T @ v [64,64] ; out = sum_c qf_c @ ctx_c ; D = qf_c . ksum_c
    out /= D

Key structure exploited: coef separates per grid axis, so
    k side carries t_b(u)  (partition-constant in spatial-major layout ->
                            2 whole-tile fused relu+scale ops)
    v side carries t_a(r)  (per-chunk partition constants -> 16 small ops)
    q side carries full coef via 4 broadcast-AP tensor_tensor ops
ksum comes from 2-col matmuls reusing the ctx weights (rhs = t_a columns).

Device layout (per core, per batch):
  - q natural channel-major [512ch, 1024n] bf16; k,v spatial-major
    [1024n, 512ch] bf16 (host pre-transpose).
  - ctx: lhsT = kb_b chunk [128tok,128ch-pair], rhs = va_a -> cps [128,4c,128]
    per pair; plus a 2-col matmul vs tg for ksum (kq).
  - out^T: lhsT = block-diag ctx [128,128] per (c,p), rhs = qm_c -> psum
    [128,512] per (pair,half), 4-branch accumulate; normalize by D_inv
    broadcast via DRAM bounce; store bf16.
  - D: kD (head-masked ksum columns) x qr -> sps; *coefD; dcomb matmul;
    reciprocal_approx on DVE.

Host converts inputs to bf16 / transposed layouts and the output back to f32
(layout/dtype prep only - all arithmetic on device).
"""

import sys

sys.path.insert(0, "/opt/trn_rl_repo")

from contextlib import ExitStack
from math import pi

import numpy as np
import ml_dtypes

from concourse import bass, mybir, tile
from concourse.bass_utils import run_bass_kernel_spmd

F32 = mybir.dt.float32
BF16 = mybir.dt.bfloat16
BFNP = ml_dtypes.bfloat16

B, C, H, W = 32, 512, 32, 32
NHEAD, MVAL = 8, 48
N = H * W            # 1024 tokens
NCORES = 8
BPC = B // NCORES    # batches per core
DN = float(C) ** -0.25
EPS = 1e-5

MULT = mybir.AluOpType.mult
ADD = mybir.AluOpType.add
MAX = mybir.AluOpType.max
COPY = mybir.ActivationFunctionType.Copy


def _fix_waits(nc):
    """This walrus build rejects sync waits on CTRL_NO-struct instructions
    (Drain/NoOp) and allows at most one on DMACopy. Hoist the excess into
    standalone InstEventSemaphore instructions on the same engine (the
    sequencer executes them in order, so semantics are unchanged)."""
    for fn in nc.m.functions:
        for bb in fn.blocks:
            newlist = []
            for inst in bb.instructions:
                si = inst.sync_info
                if isinstance(inst, mybir.InstEventSemaphore):
                    cap = 1
                elif isinstance(inst, (mybir.InstDrain, mybir.InstNoOp)):
                    cap = 0
                else:
                    cap = 1
                if si is not None and len(si.on_wait) > cap:
                    waits = list(si.on_wait)
                    for w in waits[cap:]:
                        ev = mybir.InstEventSemaphore(
                            name=nc.get_next_instruction_name(), ins=[], outs=[])
                        ev.engine = inst.engine
                        ev.sync_info = mybir.SyncInfo(on_wait=[w], on_update=[])
                        nc.register_instruction(ev, overwrite=True)
                        newlist.append(ev)
                    inst.sync_info = mybir.SyncInfo(
                        on_wait=waits[:cap], on_update=list(si.on_update))
                newlist.append(inst)
            bb.instructions[:] = newlist


def build_nc():
    nc = bass.Bass()

    qn_d = nc.declare_dram_parameter("qn", [BPC, C, N], BF16, isOutput=False)
    ks_d = nc.declare_dram_parameter("ks", [BPC, N, C], BF16, isOutput=False)
    vs_d = nc.declare_dram_parameter("vs", [BPC, N, C], BF16, isOutput=False)
    kscale_d = nc.declare_dram_parameter("kscale", [128, 2], F32, isOutput=False)
    trv_d = nc.declare_dram_parameter("trv", [128, 8, 2], F32, isOutput=False)
    tg_d = nc.declare_dram_parameter("tg", [128, 8, 2], BF16, isOutput=False)
    tab_d = nc.declare_dram_parameter("tab", [128, 4, N], BF16, isOutput=False)
    coefD_d = nc.declare_dram_parameter("coefD", [128, N], BF16, isOutput=False)
    dcomb_d = nc.declare_dram_parameter("dcomb", [128, 32], BF16, isOutput=False)
    o_d = nc.declare_dram_parameter("o", [BPC, C, N], BF16, isOutput=True)

    with tile.TileContext(nc) as tc, ExitStack() as ctx:
        const = ctx.enter_context(tc.tile_pool(name="const", bufs=1))
        stat = ctx.enter_context(tc.tile_pool(name="stat", bufs=1))
        io = ctx.enter_context(tc.tile_pool(name="io", bufs=2))
        work = ctx.enter_context(tc.tile_pool(name="work", bufs=2))
        qmp = ctx.enter_context(tc.tile_pool(name="qmp", bufs=1))
        small = ctx.enter_context(tc.tile_pool(name="small", bufs=2))
        pctx = ctx.enter_context(tc.tile_pool(name="pctx", bufs=4, space="PSUM"))
        pout = ctx.enter_context(tc.tile_pool(name="pout", bufs=2, space="PSUM"))
        psd = ctx.enter_context(tc.tile_pool(name="psd", bufs=1, space="PSUM"))
        pkq = ctx.enter_context(tc.tile_pool(name="pkq", bufs=1, space="PSUM"))
        dram = ctx.enter_context(tc.tile_pool(name="dram", bufs=2, space="DRAM"))

        kscale = const.tile([128, 2], F32, tag="kscale", name="kscale")
        trv = const.tile([128, 8, 2], F32, tag="trv", name="trv")
        tg = const.tile([128, 8, 2], BF16, tag="tg", name="tg")
        tab = const.tile([128, 4, N], BF16, tag="tab", name="tab")
        coefD = const.tile([128, N], BF16, tag="coefD", name="coefD")
        dcomb = const.tile([128, 32], BF16, tag="dcomb", name="dcomb")
        nc.sync.dma_start(kscale[:], kscale_d[:])
        nc.sync.dma_start(trv[:], trv_d[:])
        nc.sync.dma_start(tg[:], tg_d[:])
        nc.sync.dma_start(tab[:], tab_d[:])
        nc.sync.dma_start(coefD[:], coefD_d[:])
        nc.sync.dma_start(dcomb[:], dcomb_d[:])

        # persistent zero-initialized tiles; diag blocks / data cols rewritten
        # per batch, zeros never touched. Double-buffered by batch parity.
        L = {}
        kD = {}
        for par in range(2):
            for p in range(4):
                L[par, p] = stat.tile([128, 4, 128], BF16, tag=f"L{par}{p}",
                                      name=f"L{par}{p}")
                nc.gpsimd.memset(L[par, p][:].rearrange("r c f -> r (c f)"), 0.0)
                kD[par, p] = stat.tile([128, 32], BF16, tag=f"kD{par}{p}",
                                       name=f"kD{par}{p}")
                nc.gpsimd.memset(kD[par, p][:], 0.0)

        for b in range(BPC):
            par = b % 2

            qnb = io.tile([128, 4, N], BF16, tag="qnb", name="qnb")
            nc.sync.dma_start(qnb[:], qn_d[b].rearrange("(p r) n -> r p n", r=128))
            ksr = io.tile([128, 8, C], BF16, tag="ksr", name="ksr")
            nc.sync.dma_start(ksr[:], ks_d[b].rearrange("(s r) c -> r s c", r=128))
            vsr = io.tile([128, 8, C], BF16, tag="vsr", name="vsr")
            nc.sync.dma_start(vsr[:], vs_d[b].rearrange("(s r) c -> r s c", r=128))

            # k branch tensors: kb_b = relu(dn*t_b(u) * k), u == partition%32
            kb = {}
            for bb in range(2):
                t = work.tile([128, 8, C], BF16, tag=f"kb{bb}", name=f"kb{bb}")
                kb[bb] = t
                nc.vector.tensor_scalar(
                    t[:].rearrange("r s c -> r (s c)"),
                    ksr[:].rearrange("r s c -> r (s c)"),
                    kscale[:, bb:bb + 1], 0.0, MULT, MAX)

            # v branch tensors: va_a = t_a(r) * v, per-chunk partition consts.
            # a=1 built FIRST: the a=1 ctx matmul carries the psum bank-open
            # start flag and must be ready before the a=0 one (see below).
            va = {}
            for a in (1, 0):
                t = work.tile([128, 8, C], BF16, tag=f"va{a}", name=f"va{a}")
                va[a] = t
                for s in range(8):
                    nc.vector.tensor_scalar(
                        t[:, s, :], vsr[:, s, :], trv[:, s, a:a + 1], None, MULT)

            # q relu; qa = qr + eps feeds the D matmuls (the q-side eps is
            # load-bearing: some (head, token) columns relu to all-zero)
            qr = work.tile([128, 4, N], BF16, tag="qr", name="qr")
            nc.vector.tensor_scalar(
                qr[:].rearrange("r p n -> r (p n)"),
                qnb[:].rearrange("r p n -> r (p n)"), DN, 0.0, MULT, MAX)
            qa = qmp.tile([128, 4, N], BF16, tag="qa", name="qa")
            nc.vector.tensor_scalar(
                qa[:].rearrange("r p n -> r (p n)"),
                qr[:].rearrange("r p n -> r (p n)"), EPS, None, ADD)

            # qm_c = (qr + eps) * coef_c(n), coef broadcast over the 4
            # ch-blocks; built per half for SBUF residency. c = 2a + b.
            qm = {}
            for c in range(4):
                eng = nc.gpsimd if c == 3 else nc.vector
                for half in range(2):
                    t = qmp.tile([128, 4, 512], BF16, tag=f"qm{c}{half}",
                                 name=f"qm{c}{half}")
                    qm[c, half] = t
                    hs = slice(512 * half, 512 * (half + 1))
                    coefbc = (tab[:, c, hs].rearrange("r (x n) -> r x n", x=1)
                              .broadcast_to([128, 4, 512]))
                    if c == 3:
                        # gpsimd lacks scalar_tensor_tensor; qa = qr + eps
                        eng.tensor_tensor(t[:], qa[:, :, hs], coefbc, MULT)
                    else:
                        eng.scalar_tensor_tensor(
                            t[:], qr[:, :, hs], EPS, coefbc, ADD, MULT)

            # ctx phase. PSUM hazard: start_tensor_calc clears the has_written
            # bits of the WHOLE 2KB bank (zero region). Per (p, bb) phase the
            # two branch groups (a=1, a=0) share a bank: only the a=1 group
            # carries start=True and must run first (guaranteed: va1 is built
            # before va0, so its matmuls are ready earlier); the a=0 group
            # rides the same epoch - its first write lands on cleared bits
            # (overwrite), the rest accumulate.
            kq = pkq.tile([128, 4, 2, 2], F32, tag="kq", name="kq")
            cpsd = {p: pctx.tile([128, 4, 128], F32, tag="cps", name="cps")
                    for p in range(4)}
            for p in range(4):
                cs = slice(128 * p, 128 * (p + 1))
                for bb in range(2):
                    for s in range(8):
                        lhsT = kb[bb][:, s, cs]
                        for a in (1, 0):
                            nc.tensor.matmul(
                                cpsd[p][:, 2 * a + bb, :], lhsT,
                                va[a][:, s, :][:, cs],
                                start=(s == 0 and a == 1), stop=(s == 7),
                                skip_group_check=True)
                        nc.tensor.matmul(
                            kq[:, p, bb, :], lhsT, tg[:, s, :],
                            start=(s == 0), stop=(s == 7))

            # harvest: block-diag ctx -> L (lhsT of out matmuls)
            for p in range(4):
                for h in range(2):
                    rs = slice(64 * h, 64 * (h + 1))
                    nc.scalar.activation(
                        L[par, p][rs, :, rs], cpsd[p][rs, :, rs], COPY)
            # ksum -> kD head-masked columns: kD[d, 4h'+c], c = 2a+b
            for p in range(4):
                for h in range(2):
                    rs = slice(64 * h, 64 * (h + 1))
                    nc.vector.tensor_copy(
                        kD[par, p][rs, 4 * h:4 * h + 4]
                        .rearrange("r (a bb) -> r bb a", a=2),
                        kq[rs, p, :, :])

            # D: sps = kD^T @ qr (col-tiled over pairs); *coefD; dcomb matmul;
            # reciprocal on DVE (fast approx)
            dinv = small.tile([8, N], F32, tag="dinv", name="dinv")
            for half in range(2):
                hs = slice(512 * half, 512 * (half + 1))
                sps = psd.tile([128, 512], F32, tag="sps", name="sps")
                for p in range(4):
                    nc.tensor.matmul(
                        sps[32 * p:32 * (p + 1), :], kD[par, p][:],
                        qa[:, p, hs], start=True, stop=True,
                        tile_position=(0, 32 * p))
                sd = small.tile([128, 512], BF16, tag="sd", name="sd")
                nc.vector.tensor_tensor(sd[:], sps[:], coefD[:, hs], MULT)
                dps = psd.tile([32, 512], F32, tag="sps", name="dps")
                nc.tensor.matmul(dps[:], dcomb[:], sd[:], start=True, stop=True)
                with nc.allow_low_precision(reason="approx recip within tolerance"):
                    nc.vector.reciprocal(dinv[0:8, hs], dps[0:8, :])

            # broadcast D_inv rows to 64 partitions each via DRAM bounce
            dvd = dram.tile([8, N], F32, tag="dvd", name="dvd")
            nc.sync.dma_start(dvd[:], dinv[:])

            # out phase + fused normalize + store
            for p in range(4):
                for half in range(2):
                    hs = slice(512 * half, 512 * (half + 1))
                    ops = pout.tile([128, 512], F32, tag="ops", name="ops")
                    for c in range(4):
                        nc.tensor.matmul(
                            ops[:], L[par, p][:, c, :], qm[c, half][:, p, :],
                            start=(c == 0), stop=(c == 3))
                    dbc = small.tile([128, 512], F32, tag="dbc", name="dbc")
                    nc.sync.dma_start(
                        dbc[:],
                        dvd[2 * p:2 * p + 2, hs]
                        .partition_broadcast(64).rearrange("r h n -> h r n"))
                    onrm = small.tile([128, 512], BF16, tag="onrm", name="onrm")
                    nc.vector.tensor_tensor(onrm[:], ops[:], dbc[:], MULT)
                    nc.scalar.dma_start(o_d[b, 128 * p:128 * (p + 1), hs], onrm[:])

    _fix_waits(nc)
    return nc


_NC = None


def _get_nc():
    global _NC
    if _NC is None:
        _NC = build_nc()
    return _NC


def _host_prep(q, k, v, n_head, M):
    n_head = int(n_head)
    M = int(M)
    assert q.shape == (B, C, H, W) and n_head == NHEAD

    idx = np.arange(H, dtype=np.float32)
    freq = np.float32(pi / (2 * M))
    co, si = np.cos(idx * freq), np.sin(idx * freq)   # t_C, t_S over [0,32)

    # kscale[p, b] = dn * t_b(u = p % 32)
    tb = np.stack([co, si], axis=1)                    # [32, 2]
    kscale = (np.float32(DN) * np.tile(tb, (4, 1))).astype(np.float32)
    # trv[p, s, a] = t_a(r) with r = 4s + p//32
    trv = np.zeros((128, 8, 2), np.float32)
    ta = np.stack([co, si], axis=1)                    # [32, 2] over r
    for s in range(8):
        for blk in range(4):
            trv[32 * blk:32 * (blk + 1), s, :] = ta[4 * s + blk]
    tg = trv.astype(BFNP)

    # coef_c(n), c = 2a+b: t_a(r) * t_b(u); n = 32r + u
    coef = np.stack([
        np.outer(co, co), np.outer(co, si),
        np.outer(si, co), np.outer(si, si)]).reshape(4, N).astype(np.float32)
    tab = np.broadcast_to(coef[None], (128, 4, N)).astype(BFNP)

    coefD = np.zeros((128, N), np.float32)
    dcomb = np.zeros((128, 32), np.float32)
    for p in range(4):
        for h in range(2):
            for c in range(4):
                coefD[32 * p + 4 * h + c] = coef[c]
                dcomb[32 * p + 4 * h + c, 2 * p + h] = 1.0
    coefD = coefD.astype(BFNP)
    dcomb = dcomb.astype(BFNP)

    qf = q.reshape(B, C, N).astype(BFNP)
    kf = np.ascontiguousarray(
        k.reshape(B, C, N).transpose(0, 2, 1)).astype(BFNP)
    vf = np.ascontiguousarray(
        v.reshape(B, C, N).transpose(0, 2, 1)).astype(BFNP)

    in_maps = []
    for core in range(NCORES):
        b0 = core * BPC
        in_maps.append({
            "qn": qf[b0:b0 + BPC], "ks": kf[b0:b0 + BPC], "vs": vf[b0:b0 + BPC],
            "kscale": kscale, "trv": trv, "tg": tg, "tab": tab,
            "coefD": coefD, "dcomb": dcomb,
        })
    return in_maps


def run(q, k, v, n_head=8, M=48, trace=False):
    nc = _get_nc()
    in_maps = _host_prep(q, k, v, n_head, M)
    res = run_bass_kernel_spmd(nc, in_maps, core_ids=list(range(NCORES)),
                               trace=trace)
    outs = []
    for core in range(NCORES):
        o = np.asarray(res.results[core]["o"]).astype(np.float32)
        outs.append(o.reshape(BPC, C, H, W))
    return np.concatenate(outs, axis=0), res


def kernel(q, k, v, n_head=8, M=48):
    out, _ = run(q, k, v, n_head, M)
    return out
